# revision 1
# baseline (speedup 1.0000x reference)
"""Trainium2 Bass kernel for nn_CDRsAttention (sparse multi-head attention
with padding mask + CDR key mask on the first 2 heads).

Sharding: 8 cores = 4 samples (B) x 2 head-parity groups. Core (b, p)
computes heads [p, p+2, p+4, p+6] of sample b (exactly one CDR head each,
so all cores do identical work), producing a partial output
ctx_heads @ Wo_rows; the host sums the two parity partials + bo.

Host-side prep (pure numpy, cheap):
  - per-sample key gather: only keys with mask==1 participate; keys that are
    also CDR-valid (cdrs_score==1, unless the sample has no CDR tokens) come
    first, so CDR heads attend to a prefix [0, NCDR) and regular heads to
    [0, NK). Slots are zero-padded to compile-time sizes.
  - x is passed transposed with an appended ones-row (1.0 for real key
    slots, 0.0 for padding). The ones-row both applies the k/v biases through
    the projection matmuls and generates the softmax denominator: Wv is
    augmented with a selector column so v column 64 is exactly the ones-row,
    making ctx^T row 64 the softmax denominator while padded keys drop out of
    both the numerator and the denominator.

Device per core (all f32; matmuls run as float32r for full PE rate):
  qT/kT/v projections -> per head: S^T = kT_tile^T @ qT (keys on psum
  partitions), P = exp(S^T/8) on ScalarE straight out of PSUM, ctx^T
  accumulated as v_aug^T @ P, then reciprocal of the denominator row,
  a DRAM-round-trip partition broadcast, normalization multiply, and the
  output projection out = ctx_norm^T.T @ Wo_rows.
"""
import os
from contextlib import ExitStack

import ml_dtypes
import numpy as np

import concourse.bass as bass
import concourse.mybir as mybir
import concourse.tile as tile
from concourse import bacc
from concourse.bass_utils import run_bass_kernel_spmd

B, T, C, H, D = 4, 2048, 512, 8, 64
NH = 4  # heads per core
F32 = mybir.dt.float32
F32R = mybir.dt.float32r
BF16 = mybir.dt.bfloat16
EXP_SCALE = 1.0 / 8.0  # 1/sqrt(D)

_PROGRAM_CACHE: dict = {}
LAST_RESULTS = None  # BassKernelResults of the most recent kernel() call


def _chunks(total, step):
    return [(i, min(step, total - i)) for i in range(0, total, step)]


CCH = _chunks(C + 1, 128)  # contraction chunks over the 513 augmented rows


def _groups3(n):
    """Split n ktiles into groups of <=3, as even as possible."""
    ng = (n + 2) // 3
    base, rem = divmod(n, ng)
    return [base + (1 if i < rem else 0) for i in range(ng)]


def _build_program(NCDR, NK):
    NKT = NK // 128
    NKT_CDR = NCDR // 128

    nc = bacc.Bacc("TRN2", target_bir_lowering=False, debug=False, num_devices=8)
    xT_d = nc.dram_tensor("xT", [C + 1, T], BF16, kind="ExternalInput").ap()
    xkT_d = nc.dram_tensor("xkT", [C + 1, NK], BF16, kind="ExternalInput").ap()
    wq_d = nc.dram_tensor("Wq", [C + 1, 256], BF16, kind="ExternalInput").ap()
    wk_d = nc.dram_tensor("Wk", [C + 1, 256], BF16, kind="ExternalInput").ap()
    wv_d = nc.dram_tensor("Wv", [C + 1, 260], BF16, kind="ExternalInput").ap()
    wo_d = nc.dram_tensor("Wo", [256, 512], BF16, kind="ExternalInput").ap()
    out_d = nc.dram_tensor("out", [T, 512], F32, kind="ExternalOutput").ap()
    scr_d = nc.dram_tensor("scr", [32, 512], F32, kind="Internal").ap()

    with tile.TileContext(nc) as tc:
        with ExitStack() as ctx:
            _body(ctx, tc, xT_d, xkT_d, wq_d, wk_d, wv_d, wo_d, out_d, scr_d,
                  NCDR, NK, NKT, NKT_CDR)
    nc.compile()
    return nc


def _body(ctx, tc, xT_d, xkT_d, wq_d, wk_d, wv_d, wo_d, out_d, scr_d,
          NCDR, NK, NKT, NKT_CDR):
    nc = tc.nc
    Exp = mybir.ActivationFunctionType.Exp

    xpool = ctx.enter_context(tc.tile_pool(name="x", bufs=1))
    wpool = ctx.enter_context(tc.tile_pool(name="w", bufs=1))
    qkv = ctx.enter_context(tc.tile_pool(name="qkv", bufs=1))

    # ---- input loads (one big DMA per tensor + tiny aug-row DMAs) -------
    def load_split(pool, dram, rows, cols, nm):
        """[rows+1, cols] DRAM -> [128, 4*cols] main tile + [1, cols] aug tile."""
        main = pool.tile([128, 4 * cols], BF16, name=f"{nm}m", tag=f"{nm}m")
        nc.sync.dma_start(
            main[:].rearrange("p (ch c) -> p ch c", ch=4),
            dram[0:rows, :].rearrange("(ch p) c -> p ch c", p=128))
        aug = pool.tile([1, cols], BF16, name=f"{nm}a", tag=f"{nm}a")
        nc.sync.dma_start(aug[:], dram[rows:rows + 1, :])
        return [main[:, ci * cols:(ci + 1) * cols] for ci in range(4)] + [aug[:]]

    wqs = load_split(wpool, wq_d, C, 256, "wq")
    wks = load_split(wpool, wk_d, C, 256, "wk")
    wvs = load_split(wpool, wv_d, C, 260, "wv")
    wo_all = wpool.tile([128, 1024], BF16, name="wo", tag="wo")
    nc.sync.dma_start(wo_all[:].rearrange("p (g c) -> p g c", g=2),
                      wo_d[:].rearrange("(g p) c -> p g c", p=128))
    wo = [wo_all[:, 0:512], wo_all[:, 512:1024]]
    xks = load_split(xpool, xkT_d, C, NK, "xk")
    xs = load_split(xpool, xT_d, C, T, "x")

    # ---- persistent activation tiles ------------------------------------
    qT = [qkv.tile([128, T], BF16, name=f"q{p}", tag=f"q{p}") for p in range(2)]
    kT = [qkv.tile([128, NK], BF16, name=f"k{p}", tag=f"k{p}") for p in range(2)]
    v_sb = qkv.tile([128, NKT * 260], BF16, name="v", tag="v")
    ctxn = [qkv.tile([128, T], BF16, name=f"ctxn{p}", tag=f"ctxn{p}") for p in range(2)]

    nkch = _chunks(NK, 512)

    # ---- phase A: projections (PSUM->SBUF drains on ScalarE, idle here) --
    with tc.tile_pool(name="psA", bufs=3, space="PSUM") as psA, \
         tc.tile_pool(name="psAv", bufs=3, space="PSUM") as psAv:
        for p in range(2):
            for n0, ns in nkch:
                pt = psA.tile([128, ns], F32, name="pqk", tag="pqk")
                for ci, (c0, cs) in enumerate(CCH):
                    nc.tensor.matmul(
                        pt[:, :ns],
                        wks[ci][:, p * 128:(p + 1) * 128],
                        xks[ci][:, n0:n0 + ns],
                        start=(ci == 0), stop=(ci == len(CCH) - 1))
                nc.vector.tensor_copy(kT[p][:, n0:n0 + ns], pt[:, :ns])
        for kt in range(NKT):
            pt = psAv.tile([128, 260], F32, name="pv", tag="pv")
            for ci, (c0, cs) in enumerate(CCH):
                nc.tensor.matmul(
                    pt[:],
                    xks[ci][:, kt * 128:(kt + 1) * 128],
                    wvs[ci][:],
                    start=(ci == 0), stop=(ci == len(CCH) - 1))
            nc.vector.tensor_copy(v_sb[:, kt * 260:(kt + 1) * 260], pt[:])
        for p in range(2):
            pt = psA.tile([128, 512], F32, name="pqk", tag="pqk")
            for ci, (c0, cs) in enumerate(CCH):
                nc.tensor.matmul(
                    pt[:],
                    wqs[ci][:, p * 128:(p + 1) * 128],
                    xs[ci][:, 0:512],
                    start=(ci == 0), stop=(ci == len(CCH) - 1))
            nc.vector.tensor_copy(qT[p][:, 0:512], pt[:])

    # ---- phases B+C: attention + output projection ----------------------
    # Heads are processed in projection pairs (rows 0-63 / 64-127 of the
    # qT/kT pair tiles); the two K=64 score matmuls of a pair target
    # different PE row groups via tile_position and run concurrently.
    with tc.tile_pool(name="psS", bufs=2, space="PSUM") as psS, \
         tc.tile_pool(name="psCtx", bufs=2, space="PSUM") as psCtx, \
         tc.tile_pool(name="psC", bufs=2, space="PSUM") as psC, \
         tc.tile_pool(name="pP", bufs=6) as pP, \
         tc.tile_pool(name="pN", bufs=2) as pN, \
         tc.tile_pool(name="pO", bufs=3) as pO:

        def q_proj(qc, p):
            pt = psC.tile([128, 512], F32, name="qp", tag="C")
            for ci, (c0, cs) in enumerate(CCH):
                nc.tensor.matmul(
                    pt[:],
                    wqs[ci][:, p * 128:(p + 1) * 128],
                    xs[ci][:, qc * 512:(qc + 1) * 512],
                    start=(ci == 0), stop=(ci == len(CCH) - 1))
            nc.vector.tensor_copy(qT[p][:, qc * 512:(qc + 1) * 512], pt[:])

        stage = {}

        def out_proj(qc, tq):
            t0 = qc * 512 + tq * 128
            cp = psC.tile([128, 512], F32, name="C", tag="C")
            nc.tensor.matmul(cp[:], ctxn[0][:, t0:t0 + 128],
                             wo[0], start=True, stop=False)
            nc.tensor.matmul(cp[:], ctxn[1][:, t0:t0 + 128],
                             wo[1], start=False, stop=True)
            if qc not in stage:
                stage[qc] = pO.tile([128, 2048], F32, name="o", tag="o")
            nc.vector.tensor_copy(stage[qc][:, tq * 512:(tq + 1) * 512], cp[:])
            if tq == 3:
                nc.sync.dma_start(
                    out_d[qc * 512:(qc + 1) * 512, :].rearrange(
                        "(tq p) c -> p tq c", p=128),
                    stage[qc][:].rearrange("p (tq c) -> p tq c", tq=4))

        def normalize(qc, i, cu):
            idx = qc * 4 + i
            rec_row = scr_d[2 * idx + 1:2 * idx + 2, :]
            rs = pN.tile([64, 8], F32, name="rs", tag="rs")
            nc.gpsimd.dma_start(rs[:], cu[64:65, :])
            rr = pN.tile([64, 8], F32, name="rr", tag="rr")
            nc.vector.reciprocal(rr[:], rs[:])
            nc.gpsimd.dma_start(rec_row.rearrange("a (p i) -> (a p) i", p=64), rr[:])
            bc = pN.tile([64, 512], F32, name="bc", tag="bc")
            nc.gpsimd.dma_start(bc[:], rec_row.partition_broadcast(64))
            r0 = (i % 2) * 64
            q0 = qc * 512
            nc.vector.tensor_mul(
                ctxn[i // 2][r0:r0 + 64, q0:q0 + 512], cu[0:64, :], bc[:])

        for qc in range(4):
            q0 = qc * 512
            for p in range(2):
                # heads i0 = 2p (CDR when p==0), i1 = 2p+1
                nkts = [NKT_CDR if p == 0 else NKT, NKT]
                ctx_ps = [psCtx.tile([65, 512], F32, name=f"ctx{h}", tag="ctx")
                          for h in range(2)]
                done = [0, 0]
                for g0 in range(0, NKT, 2):
                    kts = [kt for kt in (g0, g0 + 1) if kt < NKT]
                    sp = [None, None]
                    # packed score matmuls: alternate row groups per ktile
                    for h in range(2):
                        mine = [kt for kt in kts if kt < nkts[h]]
                        if not mine:
                            continue
                        sp[h] = psS.tile([128, len(mine) * 512], F32,
                                         name=f"S{h}", tag="S")
                    for j, kt in enumerate(kts):
                        for h in range(2):
                            if kt < nkts[h]:
                                r0 = h * 64
                                nc.tensor.matmul(
                                    sp[h][:, j * 512:(j + 1) * 512],
                                    kT[p][r0:r0 + 64, kt * 128:(kt + 1) * 128],
                                    qT[p][r0:r0 + 64, q0:q0 + 512],
                                    start=True, stop=True,
                                    tile_position=(r0, 0))
                    for h in range(2):
                        mine = [kt for kt in kts if kt < nkts[h]]
                        if not mine:
                            continue
                        i = 2 * p + h
                        pb = pP.tile([128, len(mine) * 512], BF16,
                                     name=f"P{h}", tag="P")
                        nc.scalar.activation(pb[:], sp[h][:],
                                             Exp, scale=EXP_SCALE)
                        for j, kt in enumerate(mine):
                            nc.tensor.matmul(
                                ctx_ps[h][:],
                                v_sb[:, kt * 260 + i * 65:kt * 260 + (i + 1) * 65],
                                pb[:, j * 512:(j + 1) * 512],
                                start=(done[h] == 0),
                                stop=(done[h] + 1 == nkts[h]))
                            done[h] += 1
                for h in range(2):
                    i = 2 * p + h
                    cu = pN.tile([65, 512], F32, name=f"cu{h}", tag="cu")
                    nc.vector.tensor_copy(cu[:], ctx_ps[h][:])
                    normalize(qc, i, cu)
                # fill PE gaps: next chunk's q projection + previous
                # chunk's output projection
                if qc < 3:
                    q_proj(qc + 1, p)
                if qc > 0:
                    for tq in (0, 1) if p == 0 else (2, 3):
                        out_proj(qc - 1, tq)
        for tq in range(4):
            out_proj(3, tq)


# ---------------------------------------------------------------------------
# host side
# ---------------------------------------------------------------------------

def _round_up(n, m):
    return ((n + m - 1) // m) * m


def _host_prep(x, mask, cdrs_score, Wq, bq, Wk, bk, Wv, bv, Wo, bo):
    x = np.ascontiguousarray(np.asarray(x, np.float32))
    mask = np.asarray(mask)
    cdrs = np.asarray(cdrs_score)
    Wq = np.asarray(Wq, np.float32)
    Wk = np.asarray(Wk, np.float32)
    Wv = np.asarray(Wv, np.float32)
    Wo = np.asarray(Wo, np.float32)
    bq = np.asarray(bq, np.float32)
    bk = np.asarray(bk, np.float32)
    bv = np.asarray(bv, np.float32)

    gathers = []
    for b in range(B):
        valid = mask[b] == 1
        cdrv = valid & (cdrs[b] == 1) if np.any(cdrs[b] == 1) else valid
        regv = valid & ~cdrv
        gathers.append((np.nonzero(cdrv)[0], np.nonzero(regv)[0]))
    NCDR = max(128, _round_up(max(len(g[0]) for g in gathers), 128))
    NK = NCDR + _round_up(max(len(g[1]) for g in gathers), 128)

    # per-parity weight bundles (shared across samples)
    wbund = []
    for p in range(2):
        heads = [p, p + 2, p + 4, p + 6]
        dims = np.concatenate([np.arange(h * D, (h + 1) * D) for h in heads])
        wq_aug = np.concatenate([Wq[:, dims], bq[dims][None, :]], axis=0)
        wk_aug = np.concatenate([Wk[:, dims], bk[dims][None, :]], axis=0)
        wv_cols = []
        for h in heads:
            hd = np.arange(h * D, (h + 1) * D)
            wv = np.concatenate([Wv[:, hd], bv[hd][None, :]], axis=0)
            sel = np.zeros((C + 1, 1), np.float32)
            sel[C, 0] = 1.0
            wv_cols.append(np.concatenate([wv, sel], axis=1))
        wv_aug = np.concatenate(wv_cols, axis=1)
        wo_rows = Wo[dims, :]
        wbund.append(tuple(
            np.ascontiguousarray(w.astype(ml_dtypes.bfloat16))
            for w in (wq_aug, wk_aug, wv_aug, wo_rows)))

    in_maps = []
    for b in range(B):
        idx_cdr, idx_reg = gathers[b]
        xk = np.zeros((NK, C), np.float32)
        ones_row = np.zeros((1, NK), np.float32)
        xk[:len(idx_cdr)] = x[b, idx_cdr]
        ones_row[0, :len(idx_cdr)] = 1.0
        xk[NCDR:NCDR + len(idx_reg)] = x[b, idx_reg]
        ones_row[0, NCDR:NCDR + len(idx_reg)] = 1.0
        xT_aug = np.ascontiguousarray(
            np.concatenate([x[b].T, np.ones((1, T), np.float32)], axis=0))
        xkT_aug = np.ascontiguousarray(
            np.concatenate([xk.T, ones_row], axis=0))
        xT_bf = np.ascontiguousarray(xT_aug.astype(ml_dtypes.bfloat16))
        xkT_bf = np.ascontiguousarray(xkT_aug.astype(ml_dtypes.bfloat16))
        for p in range(2):
            wq_aug, wk_aug, wv_aug, wo_rows = wbund[p]
            in_maps.append({
                "xT": xT_bf, "xkT": xkT_bf,
                "Wq": wq_aug, "Wk": wk_aug, "Wv": wv_aug, "Wo": wo_rows,
            })
    return in_maps, NCDR, NK


def kernel(**inputs) -> np.ndarray:
    global LAST_RESULTS
    in_maps, NCDR, NK = _host_prep(**inputs)

    key = (NCDR, NK)
    nc = _PROGRAM_CACHE.get(key)
    if nc is None:
        nc = _build_program(NCDR, NK)
        _PROGRAM_CACHE[key] = nc

    res = run_bass_kernel_spmd(nc, in_maps, core_ids=list(range(8)))
    LAST_RESULTS = res

    bo = np.asarray(inputs["bo"], np.float32)
    out = np.empty((B, T, C), np.float32)
    for b in range(B):
        out[b] = res.results[2 * b]["out"] + res.results[2 * b + 1]["out"] + bo[None, :]
    return out

